# revision 26
# baseline (speedup 1.0000x reference)
"""HandGraphConvNet Trainium2 kernel.

Reference computation (eval-mode 2-layer GCN over a 21-joint hand graph):
    h  = x.reshape(S, B, 21, 2)
    h1 = relu(BN1(adj @ (h @ W1) + b1))      # hidden 21x64 per token
    h2 =      BN2(adj @ (h1 @ W2) + b2)
    out = h2.reshape(S, B, 42) + x

Kernel formulation (per core, pure data parallel over 8 cores, 4096 tokens each):
  - Fold BN scale into weights on the host:
      M1[(j,c),(i,d)] = adj[i,j] * W1[c,d] * k1[d],   k1 = g1*rsqrt(v1+eps)
      bias1[(i,d)]    = ((b1-m1)*k1+be1)[d]           (ones-row in x_aug)
      M2[(j,d),(i,e)] = adj[i,j] * W2[d,e] * k2[e]
      shift2[(i,e)]   = ((b2-m2)*k2+be2)[e]
  - Layout: channels on partitions, tokens on the free dim; all matmul operands
    fp16 (fp32 matmuls run 2-pass LOW_HIGH on trn2, ~5x slower); PSUM fp32.
  - MM1: K-tile t (128 hidden channels = 4 blocks of 32) is produced by 4
    concurrent 32-col PE tiles at row-half rp=(t+c)%2; consecutive K-tiles use
    opposite row halves so 8 PE tiles stream concurrently.
  - MM1 PSUM groups of 3 K-tiles (3 banks, [128,1536]) are evacuated with ONE
    relu instruction alternating ScalarE/VectorE (big-FD evac amortizes the
    ~400-cycle per-instruction overhead; PSUM-src is 1 elem/cycle/lane).
  - MM2 for chunk pair (2q,2q+1) accumulates 11 h1 K-tiles + a residual K-tile
    (reads x_aug with [identity; shift2]) into one PSUM bank (side0 at
    partitions 0..41, side1 at 64..105); K-steps are interleaved between MM1
    groups of the next pair to fill tensor-engine wait time.
  - DMA: x loaded as 2 big transfers on the sync HWDGE queue, weights on the
    scalar HWDGE queue (parallel ring), outputs per pair [106,512] fp32.
"""

import os
import numpy as np

import concourse.bacc as bacc
import concourse.mybir as mybir
from concourse import bass_utils
from concourse.tile import TileContext

F32 = mybir.dt.float32
F16 = mybir.dt.float16

S, B, HD = 256, 128, 42
J, CIN, HIDC = 21, 2, 64
NCORES = 8
SS = S // NCORES              # 32 seq positions per core
NTOK = SS * B                 # 4096 tokens per core
NHID = J * HIDC               # 1344 hidden channels per token
NKT = 11                      # 128-row K-tiles covering 1408 (padded hidden)
CH = 512                      # token chunk (= max psum moving free dim)
NCH = NTOK // CH              # 8 chunks
KAUG = J * CIN + 1            # 43 = 42 features + ones row
BN_EPS = 1e-5

_CACHE = {}
LAST_RESULT = None            # BassKernelResults of the most recent run (for test.py)


def _build_nc():
    nc = bacc.Bacc()
    x_ext = nc.declare_dram_parameter("x", [43, NTOK], F16, isOutput=False)
    m1_ext = nc.declare_dram_parameter("m1", [43, 44, 32], F16, isOutput=False)
    m2_ext = nc.declare_dram_parameter("m2", [128, NKT, HD], F16, isOutput=False)
    out_ext = nc.declare_dram_parameter("out", [106, NTOK // 2], F16, isOutput=True)

    relu = mybir.ActivationFunctionType.Relu

    with TileContext(nc) as tc:
        with tc.tile_pool(name="const", bufs=1) as cpool, \
             tc.tile_pool(name="xin", bufs=1) as xpool, \
             tc.tile_pool(name="h1p", bufs=8) as h1pool, \
             tc.tile_pool(name="osb", bufs=2) as opool, \
             tc.tile_pool(name="pp", bufs=3, space="PSUM") as ppool, \
             tc.tile_pool(name="po", bufs=2, space="PSUM") as popool:

            # --- ACT table preload + PE warmup, overlapped with input DMA ---
            wtile = cpool.tile([128, 128], F16)
            nc.vector.memset(wtile, 0.0)
            wtile2 = cpool.tile([64, CH], F16)
            nc.vector.memset(wtile2, 0.0)
            nc.scalar.activation(wtile[0:1, 0:8], wtile[0:1, 8:16], relu)

            m1_sb = cpool.tile([107, 44, 32], F16)
            nc.scalar.dma_start(out=m1_sb[0:43], in_=m1_ext[:])

            # x: send bare 43 feature+ones rows once over the host link; read
            # the DRAM region twice for the two partition copies. All x slices
            # ride the gpsimd (SWDGE) ring, emitted lazily (2 chunks ahead) so
            # completion sems gate only what each chunk actually needs.
            x_sb = xpool.tile([107, NTOK], F16)

            def load_x(c):
                sl = slice(c * CH, (c + 1) * CH)
                nc.gpsimd.dma_start(out=x_sb[0:43, sl], in_=x_ext[:, sl])
                nc.gpsimd.dma_start(out=x_sb[64:107, sl], in_=x_ext[:, sl])

            # x0a first on the gpsimd ring: chunk 0's first groups run
            # entirely in row-half 0 and gate on just {m1a, x0a} receipts.
            nc.gpsimd.dma_start(out=x_sb[0:43, 0:CH], in_=x_ext[:, 0:CH])
            nc.gpsimd.dma_start(out=m1_sb[64:107], in_=m1_ext[:])
            nc.gpsimd.dma_start(out=x_sb[64:107, 0:CH], in_=x_ext[:, 0:CH])
            m2_sb = cpool.tile([128, NKT, HD], F16)
            nc.scalar.dma_start(out=m2_sb, in_=m2_ext[:])
            load_x(1)

            warm = popool.tile([106, CH], F32, tag="po")
            for _ in range(2):
                nc.tensor.matmul(warm[0:32, :], wtile[0:64, 0:32],
                                 wtile2[0:64, :], start=True, stop=True)

            h1 = {}
            evac_n = [0]

            def mm1_group(c, g):
                """MM1 for chunk c, K-tiles 3g..3g+tl: fill a PSUM group tile
                then evacuate with one fused-relu instruction."""
                t0 = 2 * g
                tl = 2 if g < 5 else 1
                ps = ppool.tile([128, 2 * CH], F32, tag="ps", name=f"ps_{c}_{g}")
                for t in range(t0, t0 + tl):
                    rp = 0 if (c == 0 and g < 3) else (t + c) % 2
                    nc.tensor.matmul(
                        ps[0:128, (t - t0) * CH:(t - t0 + 1) * CH],
                        m1_sb[64 * rp:64 * rp + KAUG, 4 * t:4 * t + 4, :],
                        x_sb[64 * rp:64 * rp + KAUG, c * CH:(c + 1) * CH],
                        start=True, stop=True,
                        tile_position=(64 * rp, 0),
                    )
                dst = h1[c][:, t0 * CH:(t0 + tl) * CH]
                src = ps[:, 0:tl * CH]
                if evac_n[0] % 2 == 0:
                    nc.scalar.activation(dst, src, relu)
                else:
                    nc.vector.tensor_scalar_max(dst, src, 0.0)
                evac_n[0] += 1

            def mm2_step(q, t, po_t):
                """MM2 K-step t (0..10) for chunk pair q; shift2 rides K-tile
                10 via the constant-1.0 hidden channel at partition 64."""
                for side in (0, 1):
                    c = 2 * q + side
                    nc.tensor.matmul(
                        po_t[64 * side:64 * side + HD, :],
                        m2_sb[0:128, t, :], h1[c][:, t * CH:(t + 1) * CH],
                        start=(t == 0), stop=(t == NKT - 1),
                        tile_position=(0, 64 * side),
                    )

            # Body p: MM1 for pair p (chunks 2p,2p+1) interleaved with MM2 for
            # pair p-1. Engine queues are strict FIFO, so emission order is the
            # schedule; the weave keeps the tensor engine busy while PSUM MM1
            # groups wait on evacuation.
            WEAVE = "MMMSSSMMMSSSMMMSSSMMMSS"  # M/S in runs of 3: MM1 waves pair
            # rp0/rp1 and MM2 steps pair side0/side1 within a run; mixing
            # them serializes (array regions overlap).
            for p in range(5):
                if p < 3:
                    load_x(2 * p + 2)
                    load_x(2 * p + 3)
                items1 = []
                if p < 4:
                    for c in (2 * p, 2 * p + 1):
                        h1[c] = h1pool.tile([128, NKT * CH], F16, tag="h1", name=f"h1_{c}")
                    items1 = [(c, g) for c in (2 * p, 2 * p + 1)
                              for g in range(6)]
                po_t = None
                if p >= 1:
                    po_t = popool.tile([106, CH], F32, tag="po", name=f"po_{p - 1}")
                if p == 0:
                    for c, g in items1:
                        mm1_group(c, g)
                elif p == 4:
                    for t in range(NKT):
                        mm2_step(p - 1, t, po_t)
                else:
                    i1 = iter(items1)
                    i2 = iter(range(NKT))
                    for k in WEAVE:
                        if k == "M":
                            c, g = next(i1)
                            mm1_group(c, g)
                        else:
                            mm2_step(p - 1, next(i2), po_t)
                if po_t is not None:
                    osb = opool.tile([106, CH], F16, tag="osb", name=f"osb_{p - 1}")
                    q = p - 1
                    nc.vector.tensor_tensor(
                        osb[0:HD, :], po_t[0:HD, :],
                        x_sb[0:HD, 2 * q * CH:(2 * q + 1) * CH],
                        mybir.AluOpType.add)
                    nc.vector.tensor_tensor(
                        osb[64:64 + HD, :], po_t[64:64 + HD, :],
                        x_sb[64:64 + HD, (2 * q + 1) * CH:(2 * q + 2) * CH],
                        mybir.AluOpType.add)
                    nc.sync.dma_start(
                        out=out_ext[:, (p - 1) * CH:p * CH], in_=osb)
                    h1.pop(2 * (p - 1), None)
                    h1.pop(2 * (p - 1) + 1, None)

    nc.finalize()
    return nc


def _prep_weights(adj, W1, b1, W2, b2, g1, be1, m1, v1, g2, be2, m2, v2):
    adj = np.asarray(adj, np.float64)
    k1 = np.asarray(g1, np.float64) / np.sqrt(np.asarray(v1, np.float64) + BN_EPS)
    k2 = np.asarray(g2, np.float64) / np.sqrt(np.asarray(v2, np.float64) + BN_EPS)
    W1k = np.asarray(W1, np.float64) * k1[None, :]
    W2k = np.asarray(W2, np.float64) * k2[None, :]

    # M1[(j,c), (i,d)] = adj[i,j] * W1[c,d] * k1[d]; row j*2+c, col i*64+d
    M1 = np.einsum('ij,cd->jcid', adj, W1k).reshape(J * CIN, NHID)
    bias1 = np.tile((np.asarray(b1, np.float64) - np.asarray(m1, np.float64)) * k1
                    + np.asarray(be1, np.float64), J)
    M1a = np.concatenate([M1, bias1[None, :]], axis=0)          # (43, 1344)
    M1p = np.zeros((43, 44 * 32), np.float32)
    M1p[0:KAUG, :NHID] = M1a.astype(np.float32)
    M1p[HD, NHID] = 1.0       # constant-1.0 hidden channel (shift2 carrier)
    m1_packed = np.ascontiguousarray(M1p.reshape(43, 44, 32)).astype(np.float16)

    # M2[(j,d), (i,e)] = adj[i,j] * W2[d,e] * k2[e]; row j*64+d, col i*2+e
    M2 = np.einsum('ij,de->jdie', adj, W2k).reshape(NHID, HD)
    shift2 = np.tile((np.asarray(b2, np.float64) - np.asarray(m2, np.float64)) * k2
                     + np.asarray(be2, np.float64), J)
    m2_packed = np.zeros((128, NKT, HD), np.float32)
    M2p = np.zeros((NKT * 128, HD), np.float32)
    M2p[:NHID] = M2.astype(np.float32)
    for t in range(NKT):
        m2_packed[:, t, :] = M2p[t * 128:(t + 1) * 128, :]
    # shift2 rides the constant-1.0 hidden channel (t10 partition 64); the x
    # residual is added by VectorE at po evacuation.
    m2_packed[64, NKT - 1, :] = shift2.astype(np.float32)
    return m1_packed, m2_packed.astype(np.float16)


def kernel(x, adj, W1, b1, W2, b2, g1, be1, m1, v1, g2, be2, m2, v2):
    global LAST_RESULT
    x = np.asarray(x, np.float32)
    m1_packed, m2_packed = _prep_weights(adj, W1, b1, W2, b2,
                                         g1, be1, m1, v1, g2, be2, m2, v2)

    if "nc" not in _CACHE:
        _CACHE["nc"] = _build_nc()
    nc = _CACHE["nc"]

    in_maps = []
    for c in range(NCORES):
        xs = np.empty((KAUG, NTOK), np.float16)
        xs[0:HD] = x[c * SS:(c + 1) * SS].reshape(NTOK, HD).T
        xs[HD] = 1.0
        in_maps.append({
            "x": xs,
            "m1": m1_packed,
            "m2": m2_packed,
        })

    trace = bool(int(os.environ.get("KERNEL_TRACE", "0")))
    res = bass_utils.run_bass_kernel_spmd(
        nc, in_maps, list(range(NCORES)), trace=trace,
    )
    LAST_RESULT = res

    out = np.empty((S, B, HD), np.float32)
    for c in range(NCORES):
        oc = res.results[c]["out"].astype(np.float32)  # (106, 2048)
        ocore = np.empty((NTOK, HD), np.float32)
        for q in range(NCH // 2):
            sl = slice(q * CH, (q + 1) * CH)
            ocore[2 * q * CH:(2 * q + 1) * CH] = oc[0:HD, sl].T
            ocore[(2 * q + 1) * CH:(2 * q + 2) * CH] = oc[64:64 + HD, sl].T
        out[c * SS:(c + 1) * SS] = ocore.reshape(SS, B, HD)
    return out


# revision 28
# speedup vs baseline: 1.0400x; 1.0400x over previous
"""HandGraphConvNet Trainium2 kernel.

Reference computation (eval-mode 2-layer GCN over a 21-joint hand graph):
    h  = x.reshape(S, B, 21, 2)
    h1 = relu(BN1(adj @ (h @ W1) + b1))      # hidden 21x64 per token
    h2 =      BN2(adj @ (h1 @ W2) + b2)
    out = h2.reshape(S, B, 42) + x

Kernel formulation (per core, pure data parallel over 8 cores, 4096 tokens each):
  - Fold BN scale into weights on the host:
      M1[(j,c),(i,d)] = adj[i,j] * W1[c,d] * k1[d],   k1 = g1*rsqrt(v1+eps)
      bias1[(i,d)]    = ((b1-m1)*k1+be1)[d]           (ones-row in x_aug)
      M2[(j,d),(i,e)] = adj[i,j] * W2[d,e] * k2[e]
      shift2[(i,e)]   = ((b2-m2)*k2+be2)[e]
  - Layout: channels on partitions, tokens on the free dim; all matmul operands
    fp16 (fp32 matmuls run 2-pass LOW_HIGH on trn2, ~5x slower); PSUM fp32.
  - MM1: K-tile t (128 hidden channels = 4 blocks of 32) is produced by 4
    concurrent 32-col PE tiles at row-half rp=(t+c)%2; consecutive K-tiles use
    opposite row halves so 8 PE tiles stream concurrently.
  - MM1 PSUM groups of 3 K-tiles (3 banks, [128,1536]) are evacuated with ONE
    relu instruction alternating ScalarE/VectorE (big-FD evac amortizes the
    ~400-cycle per-instruction overhead; PSUM-src is 1 elem/cycle/lane).
  - MM2 for chunk pair (2q,2q+1) accumulates 11 h1 K-tiles + a residual K-tile
    (reads x_aug with [identity; shift2]) into one PSUM bank (side0 at
    partitions 0..41, side1 at 64..105); K-steps are interleaved between MM1
    groups of the next pair to fill tensor-engine wait time.
  - DMA: x loaded as 2 big transfers on the sync HWDGE queue, weights on the
    scalar HWDGE queue (parallel ring), outputs per pair [106,512] fp32.
"""

import os
import numpy as np

import concourse.bacc as bacc
import concourse.mybir as mybir
from concourse import bass_utils
from concourse.tile import TileContext

F32 = mybir.dt.float32
F16 = mybir.dt.float16

S, B, HD = 256, 128, 42
J, CIN, HIDC = 21, 2, 64
NCORES = 8
SS = S // NCORES              # 32 seq positions per core
NTOK = SS * B                 # 4096 tokens per core
NHID = J * HIDC               # 1344 hidden channels per token
NKT = 11                      # 128-row K-tiles covering 1408 (padded hidden)
CH = 512                      # token chunk (= max psum moving free dim)
NCH = NTOK // CH              # 8 chunks
KAUG = J * CIN + 1            # 43 = 42 features + ones row
BN_EPS = 1e-5

_CACHE = {}
LAST_RESULT = None            # BassKernelResults of the most recent run (for test.py)


def _build_nc():
    nc = bacc.Bacc()
    x_ext = nc.declare_dram_parameter("x", [43, NTOK], F16, isOutput=False)
    m1_ext = nc.declare_dram_parameter("m1", [43, 44, 32], F16, isOutput=False)
    m2_ext = nc.declare_dram_parameter("m2", [128, NKT, HD], F16, isOutput=False)
    out_ext = nc.declare_dram_parameter("out", [106, NTOK // 2], F16, isOutput=True)

    relu = mybir.ActivationFunctionType.Relu

    with TileContext(nc) as tc:
        with tc.tile_pool(name="const", bufs=1) as cpool, \
             tc.tile_pool(name="xin", bufs=1) as xpool, \
             tc.tile_pool(name="h1p", bufs=8) as h1pool, \
             tc.tile_pool(name="osb", bufs=2) as opool, \
             tc.tile_pool(name="pp", bufs=3, space="PSUM") as ppool, \
             tc.tile_pool(name="po", bufs=2, space="PSUM") as popool:

            # --- ACT table preload + PE warmup, overlapped with input DMA ---
            wtile = cpool.tile([128, 128], F16)
            nc.vector.memset(wtile, 0.0)
            wtile2 = cpool.tile([64, CH], F16)
            nc.vector.memset(wtile2, 0.0)
            nc.scalar.activation(wtile[0:1, 0:8], wtile[0:1, 8:16], relu)

            m1_sb = cpool.tile([107, 44, 32], F16)
            nc.scalar.dma_start(out=m1_sb[0:43], in_=m1_ext[:])
            nc.gpsimd.dma_start(out=m1_sb[64:107], in_=m1_ext[:])

            # x: send bare 43 feature+ones rows once over the host link; read
            # the DRAM region twice for the two partition copies. All x slices
            # ride the gpsimd (SWDGE) ring, emitted lazily (2 chunks ahead) so
            # completion sems gate only what each chunk actually needs.
            x_sb = xpool.tile([107, NTOK], F16)

            def load_x(c):
                sl = slice(c * CH, (c + 1) * CH)
                nc.gpsimd.dma_start(out=x_sb[0:43, sl], in_=x_ext[:, sl])
                nc.gpsimd.dma_start(out=x_sb[64:107, sl], in_=x_ext[:, sl])

            load_x(0)
            m2_sb = cpool.tile([128, NKT, HD], F16)
            nc.scalar.dma_start(out=m2_sb, in_=m2_ext[:])
            load_x(1)

            warm = popool.tile([106, CH], F32, tag="po")
            for _ in range(2):
                nc.tensor.matmul(warm[0:32, :], wtile[0:64, 0:32],
                                 wtile2[0:64, :], start=True, stop=True)

            h1 = {}
            evac_n = [0]

            def mm1_group(c, g):
                """MM1 for chunk c, K-tiles 3g..3g+tl: fill a PSUM group tile
                then evacuate with one fused-relu instruction."""
                t0 = 2 * g
                tl = 2 if g < 5 else 1
                ps = ppool.tile([128, 2 * CH], F32, tag="ps", name=f"ps_{c}_{g}")
                for t in range(t0, t0 + tl):
                    rp = (t + c) % 2
                    nc.tensor.matmul(
                        ps[0:128, (t - t0) * CH:(t - t0 + 1) * CH],
                        m1_sb[64 * rp:64 * rp + KAUG, 4 * t:4 * t + 4, :],
                        x_sb[64 * rp:64 * rp + KAUG, c * CH:(c + 1) * CH],
                        start=True, stop=True,
                        tile_position=(64 * rp, 0),
                    )
                dst = h1[c][:, t0 * CH:(t0 + tl) * CH]
                src = ps[:, 0:tl * CH]
                if evac_n[0] % 2 == 0:
                    nc.scalar.activation(dst, src, relu)
                else:
                    nc.vector.tensor_scalar_max(dst, src, 0.0)
                evac_n[0] += 1

            def mm2_step(q, t, po_t):
                """MM2 K-step t (0..10) for chunk pair q; shift2 rides K-tile
                10 via the constant-1.0 hidden channel at partition 64."""
                for side in (0, 1):
                    c = 2 * q + side
                    nc.tensor.matmul(
                        po_t[64 * side:64 * side + HD, :],
                        m2_sb[0:128, t, :], h1[c][:, t * CH:(t + 1) * CH],
                        start=(t == 0), stop=(t == NKT - 1),
                        tile_position=(0, 64 * side),
                    )

            # Body p: MM1 for pair p (chunks 2p,2p+1) interleaved with MM2 for
            # pair p-1. Engine queues are strict FIFO, so emission order is the
            # schedule; the weave keeps the tensor engine busy while PSUM MM1
            # groups wait on evacuation.
            WEAVE = "MMMMSSSSMMMMSSSSMMMMSSS"  # M/S in runs of 3: MM1 waves pair
            # rp0/rp1 and MM2 steps pair side0/side1 within a run; mixing
            # them serializes (array regions overlap).
            for p in range(5):
                if p < 3:
                    load_x(2 * p + 2)
                    load_x(2 * p + 3)
                items1 = []
                if p < 4:
                    for c in (2 * p, 2 * p + 1):
                        h1[c] = h1pool.tile([128, NKT * CH], F16, tag="h1", name=f"h1_{c}")
                    items1 = [(c, g) for c in (2 * p, 2 * p + 1)
                              for g in range(6)]
                po_t = None
                if p >= 1:
                    po_t = popool.tile([106, CH], F32, tag="po", name=f"po_{p - 1}")
                if p == 0:
                    for c, g in items1:
                        mm1_group(c, g)
                elif p == 4:
                    for t in range(NKT):
                        mm2_step(p - 1, t, po_t)
                else:
                    i1 = iter(items1)
                    i2 = iter(range(NKT))
                    for k in WEAVE:
                        if k == "M":
                            c, g = next(i1)
                            mm1_group(c, g)
                        else:
                            mm2_step(p - 1, next(i2), po_t)
                if po_t is not None:
                    osb = opool.tile([106, CH], F16, tag="osb", name=f"osb_{p - 1}")
                    q = p - 1
                    nc.vector.tensor_tensor(
                        osb[0:HD, :], po_t[0:HD, :],
                        x_sb[0:HD, 2 * q * CH:(2 * q + 1) * CH],
                        mybir.AluOpType.add)
                    nc.vector.tensor_tensor(
                        osb[64:64 + HD, :], po_t[64:64 + HD, :],
                        x_sb[64:64 + HD, (2 * q + 1) * CH:(2 * q + 2) * CH],
                        mybir.AluOpType.add)
                    nc.sync.dma_start(
                        out=out_ext[:, (p - 1) * CH:p * CH], in_=osb)
                    h1.pop(2 * (p - 1), None)
                    h1.pop(2 * (p - 1) + 1, None)

    nc.finalize()
    return nc


def _prep_weights(adj, W1, b1, W2, b2, g1, be1, m1, v1, g2, be2, m2, v2):
    adj = np.asarray(adj, np.float64)
    k1 = np.asarray(g1, np.float64) / np.sqrt(np.asarray(v1, np.float64) + BN_EPS)
    k2 = np.asarray(g2, np.float64) / np.sqrt(np.asarray(v2, np.float64) + BN_EPS)
    W1k = np.asarray(W1, np.float64) * k1[None, :]
    W2k = np.asarray(W2, np.float64) * k2[None, :]

    # M1[(j,c), (i,d)] = adj[i,j] * W1[c,d] * k1[d]; row j*2+c, col i*64+d
    M1 = np.einsum('ij,cd->jcid', adj, W1k).reshape(J * CIN, NHID)
    bias1 = np.tile((np.asarray(b1, np.float64) - np.asarray(m1, np.float64)) * k1
                    + np.asarray(be1, np.float64), J)
    M1a = np.concatenate([M1, bias1[None, :]], axis=0)          # (43, 1344)
    M1p = np.zeros((43, 44 * 32), np.float32)
    M1p[0:KAUG, :NHID] = M1a.astype(np.float32)
    M1p[HD, NHID] = 1.0       # constant-1.0 hidden channel (shift2 carrier)
    m1_packed = np.ascontiguousarray(M1p.reshape(43, 44, 32)).astype(np.float16)

    # M2[(j,d), (i,e)] = adj[i,j] * W2[d,e] * k2[e]; row j*64+d, col i*2+e
    M2 = np.einsum('ij,de->jdie', adj, W2k).reshape(NHID, HD)
    shift2 = np.tile((np.asarray(b2, np.float64) - np.asarray(m2, np.float64)) * k2
                     + np.asarray(be2, np.float64), J)
    m2_packed = np.zeros((128, NKT, HD), np.float32)
    M2p = np.zeros((NKT * 128, HD), np.float32)
    M2p[:NHID] = M2.astype(np.float32)
    for t in range(NKT):
        m2_packed[:, t, :] = M2p[t * 128:(t + 1) * 128, :]
    # shift2 rides the constant-1.0 hidden channel (t10 partition 64); the x
    # residual is added by VectorE at po evacuation.
    m2_packed[64, NKT - 1, :] = shift2.astype(np.float32)
    return m1_packed, m2_packed.astype(np.float16)


def kernel(x, adj, W1, b1, W2, b2, g1, be1, m1, v1, g2, be2, m2, v2):
    global LAST_RESULT
    x = np.asarray(x, np.float32)
    m1_packed, m2_packed = _prep_weights(adj, W1, b1, W2, b2,
                                         g1, be1, m1, v1, g2, be2, m2, v2)

    if "nc" not in _CACHE:
        _CACHE["nc"] = _build_nc()
    nc = _CACHE["nc"]

    in_maps = []
    for c in range(NCORES):
        xs = np.empty((KAUG, NTOK), np.float16)
        xs[0:HD] = x[c * SS:(c + 1) * SS].reshape(NTOK, HD).T
        xs[HD] = 1.0
        in_maps.append({
            "x": xs,
            "m1": m1_packed,
            "m2": m2_packed,
        })

    trace = bool(int(os.environ.get("KERNEL_TRACE", "0")))
    res = bass_utils.run_bass_kernel_spmd(
        nc, in_maps, list(range(NCORES)), trace=trace,
    )
    LAST_RESULT = res

    out = np.empty((S, B, HD), np.float32)
    for c in range(NCORES):
        oc = res.results[c]["out"].astype(np.float32)  # (106, 2048)
        ocore = np.empty((NTOK, HD), np.float32)
        for q in range(NCH // 2):
            sl = slice(q * CH, (q + 1) * CH)
            ocore[2 * q * CH:(2 * q + 1) * CH] = oc[0:HD, sl].T
            ocore[(2 * q + 1) * CH:(2 * q + 2) * CH] = oc[64:64 + HD, sl].T
        out[c * SS:(c + 1) * SS] = ocore.reshape(SS, B, HD)
    return out
